# revision 53
# baseline (speedup 1.0000x reference)
"""Multi-head attention kernel for Trainium2, SPMD across 8 NeuronCores.

Problem: b=2, s=2048, d_model=1024, 16 heads x 64 dims, packed QKV proj,
softmax over keys (boolean key mask), out-projection.

Sharding: core c in 0..7 handles batch b = c//4 and a group of 4 heads
g = c%4 (data parallel over batch x head/tensor parallel).  Each core
computes its head-group's out-projection partial [2048, 1024]; the host
sums the 4 partials per batch (the row-parallel reduction) and upcasts
from bf16.

Device-side dataflow per core (bf16 matmul operands, fp32 PSUM):
  - QKV proj, weights stationary.  Q,K produced transposed [d, s], two
    heads packed per SBUF tile (head A rows 0-63, head B rows 64-127).
    V produced in natural layout [s, d] as 16 tiles [128, 4*65] with a
    ones-column per head (col 64) that makes the PV matmul also produce
    the softmax rowsum.  Masked key rows of V (and the ones col) are
    zeroed via a per-partition scalar multiply == exact -inf masking.
    The psum->sbuf eviction and the mask multiply are fused into one
    DVE tensor_scalar_mul reading PSUM directly.
  - Scores transposed St[sk, sq]: per (pair, sq-half, key-tile) two
    [128, 1024] PSUM tiles (one per head); the two heads run as
    row-tiled concurrent matmuls (tile_position rows 0/64).  Double
    buffered so exp streams back-to-back.
  - exp SPLIT between ScalarE and DVE.  ScalarE: activation Exp
    (scale=1/8), PSUM -> bf16 SBUF.  DVE (every 3rd exp; every 8th in
    the inject loop): custom 2-instruction sequence exp(s/8) =
    p3(s/512)^64 -- a cubic-Taylor Horner op then a 6x-squaring op
    (custom DVE uOps registered at build time; max rel err ~2e-3 fp32).
  - PV: out^T[65, sq] accumulated over key tiles in PSUM; row 64 =
    rowsum.  PV trails St/exp by PIPE iterations.
  - normalize (OFF the critical path): accs are immediately evicted
    PSUM -> SBUF fp32 (releasing psB for the next j-loop), then rowsum
    row 64 DMA-hops to partition 0, gpsimd partition_broadcast to 64
    rows, reciprocal_approx_fast, multiply -> O^T packed per pair
    (head B staged through SBUF scratch + DMA into rows 64-127).  All
    of it overlaps the next j-loop's matmuls, so the PE never idles at
    j boundaries and HAM stays warm.
  - out-proj: stationary = packed O^T s-slices [128, 128], moving =
    W_out^T, both pairs accumulated in PSUM; evict split across DVE and
    ScalarE; bf16 DMA to DRAM.

Schedule notes: V projection is injected into the first attention
j-loop (PV trails by 8 there) so exp starts right after the Q/K
projections; input DMAs are emitted in first-use order and chunked so
the first matmul waits only on its own slice.  PSUM budget: 2x
[128,1024] score tiles + 2x [65,1024] accumulators = 8 banks.
"""

import numpy as np
import ml_dtypes

BF = ml_dtypes.bfloat16
S = 2048
C = 1024
DQ = 64
HL = 4  # local heads per core
KT = S // 128  # 16 key tiles
CT = C // 128  # 8 contraction tiles
SCALE = 8.0  # sqrt(DQ)

# exp(s/8) = p3(s/512)^64, p3 = cubic Taylor of e^v at 0
_R = 1.0 / 512.0
EXPC_S0 = _R * _R * _R / 6.0  # v^3 coeff (on raw scores)
EXPC_S1 = _R * _R / 2.0  # v^2 coeff
EXPC_IMM2 = _R  # v^1 coeff
# (k, head) pairs whose exp runs on DVE, per (pair, j).  ONE head per k
# (the other head's exp stays on ScalarE in parallel), kept away from the
# first/last few k so the j-boundary normalize chain and the psA ping-pong
# restart never queue behind slow DVE exps (2 x 1.22us each).  Only the
# loops without injected PE filler get DVE exps -- the padded loops are
# PE-bound and ScalarE alone keeps up.
# pair-1 j0 also carries the double-St density guard: 8 DVE exps there
# compound with the doubled score matmuls into a 2.9us/k crawl -- 4 is the
# sweet spot (measured).
DVE_EXP_KI = {
    (1, 0): {(5, 1), (7, 0), (9, 1), (11, 0)},
    (1, 1): {(5, 1), (7, 0), (9, 1), (11, 0)},
}

_CACHED = None
_DVE_OPS = None


def _register_dve_ops():
    """Register the two custom DVE exp uOps into concourse's per-process op
    table (the repo is read-only, so dve_ops.py can't be edited; appending
    to the module-level registry at runtime is equivalent -- the per-NEFF
    DVE table generator and the ISA row lookup both read these dicts)."""
    global _DVE_OPS
    if _DVE_OPS is not None:
        return _DVE_OPS
    import concourse.dve_ops as dve_ops
    from concourse.dve_ops import DveOp
    from concourse.dve_spec import Spec, Src0, C0, C1, C2, One, sq, lower
    from concourse.dve_uop import DveOpSpec

    def ref_cubic(in0, in1, s0, s1, imm2):
        x = in0.astype(np.float32)
        return (
            (np.float32(s0) * x + np.float32(s1)) * x + np.float32(imm2)
        ) * x + np.float32(1.0)

    def ref_pow64(in0, in1, s0, s1, imm2):
        x = in0.astype(np.float32)
        for _ in range(6):
            x = (x * x).astype(np.float32)
        return x

    body1 = ((C0 * Src0 + C1) * Src0 + C2) * Src0 + One
    b = Src0
    for _ in range(6):
        b = sq(b)
    specs = [
        ("EXP_CUBIC_ANTK", Spec(body=body1, reference=ref_cubic)),
        ("POW64_ANTK", Spec(body=b, reference=ref_pow64)),
    ]
    ops = []
    for name, spec in specs:
        if name in dve_ops._SUB_OPCODE_FOR_NAME:
            ops.append(next(o for o in dve_ops.OPS if o.name == name))
            continue
        row = dve_ops._CUSTOM_DVE_ROW_BASE + len(dve_ops.OPS)
        assert row < 0x20
        shas = {}
        for ver in ("v3", "v4"):
            s = DveOpSpec(
                name=name, opcode=row, uops=lower(spec, ver=ver), rd1_en=False
            )
            shas[ver] = s.sha(ver)
        op = DveOp(name, spec, subdim=False, uops_sha=shas)
        dve_ops.OPS.append(op)
        dve_ops._SUB_OPCODE_FOR_NAME[name] = row
        dve_ops.CUSTOM_DVE_SPECS[name] = spec
        ops.append(op)
    _DVE_OPS = tuple(ops)
    return _DVE_OPS


def _build():
    import concourse.bacc as bacc
    import concourse.mybir as mybir
    import concourse.tile as tile

    EXP_CUBIC, POW64 = _register_dve_ops()

    F32 = mybir.dt.float32
    BF16 = mybir.dt.bfloat16
    EXP = mybir.ActivationFunctionType.Exp

    nc = bacc.Bacc(
        "TRN2",
        target_bir_lowering=False,
        debug=False,
        enable_asserts=False,
        num_devices=8,
    )

    XT = nc.dram_tensor("xt", [C, S], BF16, kind="ExternalInput").ap()
    WQ = nc.dram_tensor("wq", [128, CT * 256], BF16, kind="ExternalInput").ap()
    WK = nc.dram_tensor("wk", [128, CT * 256], BF16, kind="ExternalInput").ap()
    WV = nc.dram_tensor("wv", [C, 2 * 128], BF16, kind="ExternalInput").ap()
    WO = nc.dram_tensor("wo", [HL * DQ, C], BF16, kind="ExternalInput").ap()
    MV = nc.dram_tensor("maskv", [128, KT], F32, kind="ExternalInput").ap()
    OUT = nc.dram_tensor("out", [S, C], BF16, kind="ExternalOutput").ap()

    from contextlib import ExitStack

    with tile.TileContext(nc) as tc:
        with ExitStack() as stack:
            pool = lambda *a, **k: stack.enter_context(tc.tile_pool(*a, **k))
            p_xt = pool(name="xt", bufs=CT)
            p_w = pool(name="wqk", bufs=2)
            p_wv = pool(name="wv", bufs=CT)
            p_wo = pool(name="wo", bufs=2)
            p_c = pool(name="cst", bufs=1)
            p_qk = pool(name="qk", bufs=4)
            p_v = pool(name="v", bufs=KT)
            p_pt = pool(name="pt", bufs=18)
            p_h = pool(name="h", bufs=2)
            p_ev = pool(name="ev", bufs=4)
            p_rc = pool(name="rc", bufs=2)
            p_rs = pool(name="rs", bufs=2)
            p_bc = pool(name="bc", bufs=1)
            p_ot = pool(name="ot", bufs=4)
            p_sc = pool(name="sc", bufs=1)
            p_os = pool(name="os", bufs=4)
            psA = pool(name="psA", bufs=2, space="PSUM")
            psB = pool(name="psB", bufs=2, space="PSUM")
            # ---------------- input DMA ----------------
            # Order matters: the first projection matmul waits on wq + xt[0],
            # so emit those DMAs first; wv/wo/mask are needed much later.
            wq_sb = p_w.tile([128, CT * 256], BF16, tag="wq", name="wq_sb")
            wk_sb = p_w.tile([128, CT * 256], BF16, tag="wk", name="wk_sb")
            xt_t = [p_xt.tile([128, S], BF16, tag="xt", name="xt_t") for _ in range(CT)]
            # chunked so the first projection matmuls wait only on their
            # slice; the sq-half-1 X columns are only needed by the
            # injected half-projections, so they stream in AFTER everything
            # the lead-in Q/K half-0 projections touch (~3MB instead of 9MB
            # ahead of the first score matmul)
            for c in range(CT):
                nc.sync.dma_start(
                    wq_sb[:, c * 256 : (c + 1) * 256], WQ[:, c * 256 : (c + 1) * 256]
                )
                nc.sync.dma_start(
                    xt_t[c][:, 0:1024], XT[c * 128 : (c + 1) * 128, 0:1024]
                )
                nc.sync.dma_start(
                    wk_sb[:, c * 256 : (c + 1) * 256], WK[:, c * 256 : (c + 1) * 256]
                )
            for c in range(CT):
                nc.sync.dma_start(
                    xt_t[c][:, 1024:2048], XT[c * 128 : (c + 1) * 128, 1024:2048]
                )
            wv_t = []
            for c in range(CT):
                t = p_wv.tile([128, HL * DQ], BF16, tag="wv", name="wv_t")
                nc.sync.dma_start(t[:], WV[c * 128 : (c + 1) * 128, :])
                wv_t.append(t)
            mv_t = p_c.tile([128, KT + 8], F32, tag="mv", name="mv_t")
            nc.sync.dma_start(mv_t[:, 0:KT], MV[:])
            # ones scratch for the V ones-column (written once)
            nc.vector.memset(mv_t[:, KT : KT + 4], 1.0)
            ones64 = p_c.tile([1, 64], F32, tag="ones64", name="ones64")
            nc.vector.memset(ones64[0:1, 0:64], 1.0)
            # dummy exp to pull the ScalarE ACT-table load (~2.7us) into the
            # DMA lead-in instead of delaying the first real exp
            nc.scalar.activation(
                mv_t[0:1, KT + 5 : KT + 6], mv_t[0:1, KT : KT + 1], EXP
            )
            wo_t = []
            for p in range(2):
                t = p_wo.tile([128, C], BF16, tag="wo", name="wo_t")
                nc.sync.dma_start(t[:], WO[p * 128 : (p + 1) * 128, :])
                wo_t.append(t)

            # ---------------- QKV projection ----------------
            qk_tiles = {}

            def proj_qk_half(nm, wsb, pair, pools, half):
                # one sq-half of a Q/K projection: small enough (16 MMs,
                # ~3.4us) that injecting it mid-j-loop only causes a short
                # exp drought (the 2-tile exp backlog covers most of it)
                key = (nm, pair)
                if key not in qk_tiles:
                    qk_tiles[key] = p_qk.tile([128, S], BF16, tag="qk", name="qk_t")
                dst = qk_tiles[key]
                off = half * 1024
                pst = pools[0].tile([128, 1024], F32, tag=pools[1], name="pp")
                for c in range(CT):
                    wt = wsb[:, c * 256 + pair * 128 : c * 256 + (pair + 1) * 128]
                    for n in range(2):
                        nc.tensor.matmul(
                            pst[:, n * 512 : (n + 1) * 512],
                            lhsT=wt,
                            rhs=xt_t[c][:, off + n * 512 : off + (n + 1) * 512],
                            start=(c == 0),
                            stop=(c == CT - 1),
                        )
                nc.vector.tensor_copy(dst[:, off : off + 1024], pst[:, 0:1024])

            def proj_qk(nm, wsb, pair, pools):
                proj_qk_half(nm, wsb, pair, pools, 0)
                proj_qk_half(nm, wsb, pair, pools, 1)

            def proj_v_tile(st):
                psv = psB.tile([128, HL * DQ], F32, tag="B", name="psv")
                for c in range(CT):
                    nc.tensor.matmul(
                        psv[:, 0 : HL * DQ],
                        lhsT=xt_t[c][:, st * 128 : (st + 1) * 128],
                        rhs=wv_t[c][:],
                        start=(c == 0),
                        stop=(c == CT - 1),
                    )
                vt = p_v.tile([128, HL * 65], BF16, tag="v", name="v_t")
                v3 = vt[:, 0 : HL * 65].rearrange("p (h c) -> p h c", c=65)
                s3 = psv[:, 0 : HL * DQ].rearrange("p (h c) -> p h c", c=DQ)
                o3 = mv_t[:, KT : KT + 4].rearrange("p (h c) -> p h c", c=1)
                # fused eviction+mask: V cols straight from PSUM * mask, and
                # the ones-column = 1.0 * mask
                nc.vector.tensor_scalar_mul(
                    v3[:, :, 0:DQ], s3[:, :, :], mv_t[:, st : st + 1]
                )
                nc.vector.tensor_scalar_mul(
                    v3[:, :, DQ : DQ + 1], o3[:, :, :], mv_t[:, st : st + 1]
                )
                v_t.append(vt)

            v_t = []
            # Lead-in projects ONLY the sq-half-0 of pair-0 Q and K: that is
            # all j-loop 0 needs for its first 8 key tiles, so the first
            # score matmul fires after ~3MB of DMA + 32 MMs.  All remaining
            # projection halves (K0h1, Q0h1, V, and pair-1 Q/K) are injected
            # INSIDE the exp-paced attention loops, where they double as PE
            # density filler that keeps the HAM clock-gate at full rate.
            proj_qk_half("q", wq_sb, 0, (psA, "A"), 0)
            proj_qk_half("k", wk_sb, 0, (psB, "B"), 0)
            # discarded warmup matmuls: fill the DMA-paced gaps at the entry
            # to the exp-paced attention stream so the HAM clock-gate sees a
            # continuous busy window and commits to full rate
            wup = psB.tile([128, 512], F32, tag="B", name="wup")
            for r in range(2):
                for c in range(CT):
                    nc.tensor.matmul(
                        wup[:, 0:512],
                        lhsT=wq_sb[:, c * 256 : c * 256 + 128],
                        rhs=xt_t[c][:, 0:512],
                        start=True,
                        stop=True,
                    )

            # ---------------- attention ----------------
            # Per (pair, j-half): ping-pong St tiles [128,1024] per head so
            # exp streams back-to-back while the PE computes the next scores;
            # PV trails PIPE iterations behind so a blocked acc slot at a
            # j-boundary doesn't head-of-line-block St in the PE FIFO.
            PIPE = 3
            ot_halves = {}  # (pair, j) -> [128, 1024] bf16 packed O^T
            scr = p_sc.tile([64, S], BF16, tag="sc", name="sc_t")

            def emit_outproj_st(st, pool, tag):
                po = pool.tile([128, C], F32, tag=tag, name="po")
                sj, so = st // 8, (st % 8) * 128
                for p in range(2):
                    for n in range(2):
                        nc.tensor.matmul(
                            po[:, n * 512 : (n + 1) * 512],
                            lhsT=ot_halves[(p, sj)][:, so : so + 128],
                            rhs=wo_t[p][:, n * 512 : (n + 1) * 512],
                            start=(p == 0),
                            stop=(p == 1),
                        )
                os_t = p_os.tile([128, C], BF16, tag="os", name="os_t")
                nc.vector.tensor_copy(os_t[:, 0:512], po[:, 0:512])
                if st < 8:
                    # injected slices: ONE whole-tile DMA on sync -- extra
                    # DMA instructions here congest the sync queue that the
                    # concurrent normalize chain depends on
                    nc.scalar.copy(os_t[:, 512:1024], po[:, 512:1024])
                    nc.sync.dma_start(OUT[st * 128 : (st + 1) * 128, :], os_t[:])
                else:
                    # tail slices (ScalarE idle by then): stripe the output
                    # DMAs across BOTH hardware DGE paths (sync + scalar) --
                    # a single queue serializes the 2MB tail drain
                    dma0 = nc.scalar if st % 2 == 0 else nc.sync
                    dma1 = nc.scalar if st % 2 == 1 else nc.sync
                    dma0.dma_start(
                        OUT[st * 128 : (st + 1) * 128, 0:512], os_t[:, 0:512]
                    )
                    nc.scalar.copy(os_t[:, 512:1024], po[:, 512:1024])
                    dma1.dma_start(
                        OUT[st * 128 : (st + 1) * 128, 512:1024], os_t[:, 512:1024]
                    )

            def emit_exp(pt, stp, k, i, dve_set):
                if (k, i) in dve_set:
                    h = p_h.tile([128, 1024], F32, tag="h", name="h_t")
                    nc.vector._custom_dve(
                        EXP_CUBIC,
                        out=h[:],
                        in0=stp[:],
                        s0=EXPC_S0,
                        s1=EXPC_S1,
                        imm2=EXPC_IMM2,
                    )
                    nc.vector._custom_dve(POW64, out=pt[:], in0=h[:])
                else:
                    nc.scalar.activation(pt[:], stp[:], EXP, scale=1.0 / SCALE)

            def attention(pair):
                qt = qk_tiles[("q", pair)]
                kt = qk_tiles[("k", pair)]
                hA, hB = 2 * pair, 2 * pair + 1
                for j in range(2):
                    ot = p_ot.tile([128, 1024], BF16, tag="ot", name="ot_t")
                    ot_halves[(pair, j)] = ot
                    # PE-filler injection keeps every exp-paced loop dense so
                    # the HAM clock-gate stays at full rate: pair-0 j0 gets
                    # the V projection (PV trails by 8 there -- it needs V),
                    # pair-0 j1 gets the pair-1 Q/K projections, pair-1 j1
                    # gets out-proj s-slices 0-7.
                    inject = pair == 0 and j == 0
                    pipe = 8 if inject else PIPE
                    dve_set = DVE_EXP_KI.get((pair, j), set())
                    jo = j * 1024
                    accs = []
                    pts = {}

                    # pair-1 j0 is the one loop with no PE filler available
                    # (its own outputs gate everything injectable): compute
                    # each score tile TWICE -- the first pass is discarded
                    # (start=True resets PSUM) and exists only to keep the PE
                    # dense enough that the HAM clock-gate never throttles it
                    # to half rate (a stuck-cold window costs ~12us here).
                    # Same guard for inject-j0's first k's (before the V
                    # projection provides density).
                    def reps_for(k):
                        if pair == 1 and j == 0:
                            return 2
                        if inject and k < 7:
                            return 2
                        return 1

                    def st_exp(k):
                        for i, base in enumerate((0, 64)):
                            stp = psA.tile([128, 1024], F32, tag="A", name="stp")
                            for rep in range(reps_for(k)):
                                for n in range(2):
                                    nc.tensor.matmul(
                                        stp[:, n * 512 : (n + 1) * 512],
                                        lhsT=kt[base : base + DQ, k * 128 : (k + 1) * 128],
                                        rhs=qt[base : base + DQ, jo + n * 512 : jo + (n + 1) * 512],
                                        start=True,
                                        stop=True,
                                    )
                            pt = p_pt.tile([128, 1024], BF16, tag="pt", name="pt_t")
                            emit_exp(pt, stp, k, i, dve_set)
                            pts[(k, i)] = pt

                    def pv(k):
                        if not accs:
                            accs.append(psB.tile([65, 1024], F32, tag="B", name="acc"))
                            accs.append(psB.tile([65, 1024], F32, tag="B", name="acc"))
                        for i, h in enumerate((hA, hB)):
                            pt = pts.pop((k, i))
                            for n in range(2):
                                nc.tensor.matmul(
                                    accs[i][0:65, n * 512 : (n + 1) * 512],
                                    lhsT=v_t[k][:, h * 65 : h * 65 + 65],
                                    rhs=pt[:, n * 512 : (n + 1) * 512],
                                    start=(k == 0),
                                    stop=(k == KT - 1),
                                )

                    for k in range(KT):
                        st_exp(k)
                        if inject:
                            if k == 2:
                                # key tiles 8-15 need kt cols 1024-2047
                                proj_qk_half("k", wk_sb, 0, (psA, "A"), 1)
                            elif k == 5:
                                # j-loop 1 needs qt cols 1024-2047
                                proj_qk_half("q", wq_sb, 0, (psA, "A"), 1)
                            elif k == 7:
                                for st in range(KT):
                                    proj_v_tile(st)
                        if pair == 0 and j == 1:
                            if k == 1:
                                proj_qk_half("q", wq_sb, 1, (psA, "A"), 0)
                            elif k == 4:
                                proj_qk_half("q", wq_sb, 1, (psA, "A"), 1)
                            elif k == 8:
                                proj_qk_half("k", wk_sb, 1, (psA, "A"), 0)
                            elif k == 11:
                                proj_qk_half("k", wk_sb, 1, (psA, "A"), 1)
                        if pair == 1 and j == 1 and 6 <= k <= 13:
                            emit_outproj_st(k - 6, psA, "A")
                        if k >= pipe:
                            pv(k - pipe)
                    for k in range(KT - pipe, KT):
                        pv(k)

                    # normalize: O = PV / rowsum (rowsum in acc row 64).
                    # DVE does [evict A, evict B, recip A, recip B] back to
                    # back (each recip's input is the just-finished eviction,
                    # so the DVE FIFO head never waits cross-engine); psB is
                    # free for the next j right after the evictions.  The
                    # RECIPROCAL row then DMA-hops to partition 0 and gpsimd
                    # broadcasts + multiplies -- all waits land on the
                    # otherwise-idle gpsimd/DMA queues, overlapping the next
                    # j-loop entirely.
                    evs = []
                    rs = p_rs.tile([1, 2048], F32, tag="rs", name="rs_t")
                    for i in range(2):
                        ev = p_ev.tile([65, 1024], F32, tag="ev", name="ev_t")
                        nc.vector.tensor_copy(ev[0:65, 0:1024], accs[i][0:65, 0:1024])
                        evs.append(ev)
                        nc.sync.dma_start(
                            rs[0:1, i * 1024 : (i + 1) * 1024], ev[64:65, 0:1024]
                        )
                    # ONE reciprocal + ONE broadcast for both heads (a second
                    # back-to-back partition_broadcast costs a ~2.6us gpsimd
                    # pipeline drain; gpsimd runs ONLY partition_broadcast so
                    # its ucode library never swaps)
                    rc = p_rc.tile([1, 2048], F32, tag="rc", name="rc_t")
                    nc.vector.reciprocal_approx_fast(
                        rc[0:1, 0:2048], rs[0:1, 0:2048]
                    )
                    # head A -> rows 0-63; head B -> rows 64-127 directly (a
                    # 64-partition DVE op may write the other half-quadrant
                    # pair: bank0->Q2, bank1->Q3)
                    dsts = (ot[0:64, 0:1024], ot[64:128, 0:1024])
                    if pair == 1 and j == 1:
                        # last boundary is ON the critical path to out-proj
                        # st8-15 and PSUM is free: broadcast via two tiny PE
                        # matmuls (ones[1,64]^T @ rc) instead of the ~8us
                        # gpsimd partition_broadcast + drain
                        for i in range(2):
                            bcp = psB.tile([64, 1024], F32, tag="B", name="bcp")
                            for n in range(2):
                                nc.tensor.matmul(
                                    bcp[0:64, n * 512 : (n + 1) * 512],
                                    lhsT=ones64[0:1, 0:64],
                                    rhs=rc[0:1, i * 1024 + n * 512 : i * 1024 + (n + 1) * 512],
                                    start=True,
                                    stop=True,
                                )
                            nc.vector.tensor_mul(
                                dsts[i], evs[i][0:64, 0:1024], bcp[0:64, 0:1024]
                            )
                    else:
                        bct = p_bc.tile([64, 2048], F32, tag="bc", name="bc_t")
                        nc.gpsimd.partition_broadcast(
                            bct[0:64, 0:2048], rc[0:1, 0:2048]
                        )
                        for i in range(2):
                            nc.vector.tensor_mul(
                                dsts[i],
                                evs[i][0:64, 0:1024],
                                bct[0:64, i * 1024 : (i + 1) * 1024],
                            )

            attention(0)
            attention(1)

            # ---------------- out-projection (s-slices 8-15) ----------------
            # slices 0-7 were injected into pair-1 j-loop 1 above
            ps_cycle = [(psA, "A"), (psB, "B")]
            for st in range(8, KT):
                pool, tag = ps_cycle[st % 2]
                emit_outproj_st(st, pool, tag)

    nc.compile()
    return nc


def _get_nc():
    global _CACHED
    if _CACHED is None:
        _CACHED = _build()
    return _CACHED


def _prep_in_maps(X, W_qkv, W_out, mask):
    X = np.asarray(X, dtype=np.float32)
    Wqkv = np.asarray(W_qkv, dtype=np.float32)
    Wo = np.asarray(W_out, dtype=np.float32)
    m = np.asarray(mask)
    W3 = Wqkv.reshape(16, DQ, 3, C)
    in_maps = []
    for core in range(8):
        b = core // 4
        g = core % 4
        hs = slice(4 * g, 4 * g + 4)
        wq = W3[hs, :, 0, :].reshape(HL * DQ, C).T.astype(BF)
        wk = W3[hs, :, 1, :].reshape(HL * DQ, C).T.astype(BF)
        # pre-arrange for contiguous SBUF prestage: [128, c*256+j]
        wq = np.ascontiguousarray(
            wq.reshape(CT, 128, HL * DQ).transpose(1, 0, 2).reshape(128, CT * 256)
        )
        wk = np.ascontiguousarray(
            wk.reshape(CT, 128, HL * DQ).transpose(1, 0, 2).reshape(128, CT * 256)
        )
        wv = np.ascontiguousarray(W3[hs, :, 2, :].reshape(HL * DQ, C).T.astype(BF))
        wo = np.ascontiguousarray(Wo[:, 256 * g : 256 * (g + 1)].T.astype(BF))
        xt = np.ascontiguousarray(X[b].T.astype(BF))
        mv = np.ascontiguousarray(
            m[b].astype(np.float32).reshape(KT, 128).T
        )
        in_maps.append(
            {"xt": xt, "wq": wq, "wk": wk, "wv": wv, "wo": wo, "maskv": mv}
        )
    return in_maps


def _run(in_maps, trace=False, **kw):
    from concourse import bass_utils

    nc = _get_nc()
    return bass_utils.run_bass_kernel_spmd(
        nc, in_maps, core_ids=list(range(8)), trace=trace, **kw
    )


def _gather(results):
    out = np.empty((2, S, C), dtype=np.float32)
    p = [r["out"].astype(np.float32) for r in results]
    out[0] = p[0] + p[1] + p[2] + p[3]
    out[1] = p[4] + p[5] + p[6] + p[7]
    return out


def kernel(X, W_qkv, W_out, mask):
    in_maps = _prep_in_maps(X, W_qkv, W_out, mask)
    res = _run(in_maps)
    return _gather(res.results)
